# revision 1
# baseline (speedup 1.0000x reference)
"""Trainium2 Bass kernel for nn_BatchMultiHeadGraphAttention (v2.1).

Math: out[b,c,h] = softmax_j(mask(leaky(src_i + dst_j))) @ Hm  where
Hm = h[b,c] @ w[c,h], t = tanh(Hm), src = t @ a_src, dst = t @ a_dst.

Identity: exp(leaky(x)) = max(e^x, e^{0.2x}), both branches rank-1 in
(i,j).  With s_ij = 1{src_i >= -dst_j}, Vp = V .* s (V = adj+selfloops):
  num = Vp @ (b .* Haug) + r_i * ( V @ (d .* Haug) - Vp @ (d .* Haug) )
  b_j = e^{dst_j}, d_j = e^{0.2 dst_j}, r_i = e^{-0.8 src_i},
  Haug = [Hm | 1];  out = num[:, :64] / num[:, 64].

Sharding: core = b*2 + cpair; each core does one b and two c's (4 heads).
Schedule: stage A (Hm, tanh, attn vectors, Hbd) for both c's up front,
then the 8 (c,h) mask+matmul+combine pipelines interleaved.
"""

import os
import sys
from contextlib import ExitStack

import numpy as np
import ml_dtypes

sys.path.insert(0, "/opt/trn_rl_repo")

import concourse.bass as bass
import concourse.bacc as bacc
import concourse.tile as tile
from concourse import mybir
from concourse.masks import make_identity
from concourse.bass_utils import run_bass_kernel_spmd

F32 = mybir.dt.float32
BF16 = mybir.dt.bfloat16
AF = mybir.ActivationFunctionType
OP = mybir.AluOpType

N = 1024
NB = 8
F = 64
C2 = 2
NH = 4

MASK_MODE = os.environ.get("MASK_MODE", "stt")
HBD_ENGINE = os.environ.get("HBD_ENGINE", "pool")
STAGE_ENGINE = os.environ.get("STAGE_ENGINE", "pool")


def build_kernel(nc: bass.Bass, tc: tile.TileContext, ctx: ExitStack, ins, out_ap):
    vT_ap = ins["vT"]
    hTe_ap = ins["hTe"]
    we_ap = ins["we"]
    wb_ap = ins["wb"]
    aab_ap = ins["aab"]

    # ---------------- pools ----------------
    constp = ctx.enter_context(tc.tile_pool(name="const", bufs=1))
    apool = ctx.enter_context(tc.tile_pool(name="apool", bufs=1))
    vppool = ctx.enter_context(tc.tile_pool(name="vppool", bufs=4))
    smallp = ctx.enter_context(tc.tile_pool(name="smallp", bufs=2))
    vecp = ctx.enter_context(tc.tile_pool(name="vecp", bufs=1))
    sbcp = ctx.enter_context(tc.tile_pool(name="sbcp", bufs=8))
    pshm = ctx.enter_context(tc.tile_pool(name="pshm", bufs=1, space="PSUM"))
    psvec = ctx.enter_context(tc.tile_pool(name="psvec", bufs=1, space="PSUM"))
    psov = ctx.enter_context(tc.tile_pool(name="psov", bufs=2, space="PSUM"))
    pspp = ctx.enter_context(tc.tile_pool(name="pspp", bufs=4, space="PSUM"))
    dramp = ctx.enter_context(tc.tile_pool(name="dramp", bufs=1, space="DRAM"))

    # ---------------- constants ----------------
    vT = constp.tile([128, NB, N], BF16)
    hTe = constp.tile([65, C2, N], BF16)
    wb = constp.tile([64, C2, NH, F], BF16)
    aab = constp.tile([128, C2, 2, 4], BF16)
    we = constp.tile([65, C2, NH * 65], BF16)
    nc.sync.dma_start(out=vT[:, 0, :], in_=vT_ap[0:128, :])
    nc.sync.dma_start(out=hTe[:], in_=hTe_ap[:])
    nc.sync.dma_start(out=wb[:], in_=wb_ap[:])
    nc.sync.dma_start(out=aab[:], in_=aab_ap[:])
    nc.sync.dma_start(out=we[:], in_=we_ap[:])
    for jb in range(1, NB):
        nc.sync.dma_start(out=vT[:, jb, :], in_=vT_ap[jb * 128:(jb + 1) * 128, :])

    # ---------------- persistent A-stage outputs ----------------
    rcol_sb = {}
    H_aug = apool.tile([128, C2, NB, NH, 65], BF16)
    Hbd = apool.tile([128, C2, NB, NH, 130], BF16)
    srcb_l = [apool.tile([4, 2, N], BF16, tag=f"srcb{i}", name=f"srcb{i}")
              for i in range(C2)]
    srd = dramp.tile([C2, 4, 2, N], BF16, tag="srd")
    dstneg_l = [apool.tile([128, NB], F32, tag=f"dn{i}", name=f"dn{i}") for i in range(8)]
    negr_l = [apool.tile([128, NB], F32, tag=f"nr{i}", name=f"nr{i}") for i in range(8)]
    ident = constp.tile([128, 128], BF16)
    make_identity(nc, ident[:])

    sbc_l = {}
    vpt_l = {}

    def make_sbc(gh):
        c, h = gh // NH, gh % NH
        hp, hr = h // 2, h % 2
        sbc = sbcp.tile([128, N], BF16, tag="sbc")
        nc.gpsimd.dma_start(
            out=sbc[:],
            in_=srd[c, 2 * hr:2 * hr + 1, hp, :].to_broadcast([128, N]),
        )
        sbc_l[gh] = sbc

    # =================== stage A (both c) ===================
    for c in range(C2):
        # ---- A2: tTb = tanh(Hm).T per head pair [128, hp, N] bf16 ----
        tTb = smallp.tile([128, 2, N], BF16, tag="ttb")
        for hp in range(2):
            for nh in range(2):
                pht = pshm.tile([128, 512], F32, tag="ph")
                nc.tensor.matmul(
                    pht[:],
                    lhsT=wb[:, c, 2 * hp:2 * hp + 2, :],
                    rhs=hTe[0:64, c, nh * 512:(nh + 1) * 512],
                    start=True,
                    stop=True,
                )
                nc.scalar.activation(
                    out=tTb[:, hp, nh * 512:(nh + 1) * 512], in_=pht[:],
                    func=AF.Tanh,
                )

        # ---- A3: src/dst column layout via block-diag vectors ----
        # sv[:, hp, nb, k], k = (src_h0, negdst_h0, src_h1, negdst_h1)
        psv = psvec.tile([128, 2, NB, 4], F32, tag="psv")
        for hp in range(2):
            for nb in range(NB):
                nc.tensor.matmul(
                    psv[:, hp, nb, :],
                    lhsT=tTb[:, hp, nb * 128:(nb + 1) * 128],
                    rhs=aab[:, c, hp, :],
                    start=True,
                    stop=True,
                )
        sv = vecp.tile([128, 2, NB, 4], F32, tag="sv")
        nc.scalar.activation(out=sv[:], in_=psv[:], func=AF.Copy)

        # compact per-head columns: dstneg (mask scalar), rcol, bcol, dcol
        bcol = vecp.tile([128, NH, NB], F32, tag="bcol")
        dcol = vecp.tile([128, NH, NB], F32, tag="dcol")
        rcol = apool.tile([128, NH, NB], F32, tag=f"rc{c}")
        rcol_sb[c] = rcol
        for h in range(NH):
            hp, hr = h // 2, h % 2
            nc.scalar.activation(
                out=dstneg_l[c * NH + h][:], in_=sv[:, hp, :, 2 * hr + 1],
                func=AF.Copy,
            )
            nc.scalar.activation(
                out=rcol[:, h, :], in_=sv[:, hp, :, 2 * hr], func=AF.Exp,
                scale=-0.8,
            )
            nc.vector.tensor_scalar(
                out=negr_l[c * NH + h][:], in0=rcol[:, h, :], scalar1=-1.0,
                scalar2=None, op0=OP.mult,
            )
            nc.scalar.activation(
                out=bcol[:, h, :], in_=sv[:, hp, :, 2 * hr + 1], func=AF.Exp,
                scale=-1.0,
            )
            nc.scalar.activation(
                out=dcol[:, h, :], in_=sv[:, hp, :, 2 * hr + 1], func=AF.Exp,
                scale=-0.2,
            )

        # ---- A4: src row layout -> DRAM (for free-dim broadcast) ----
        srcb = srcb_l[c]
        for hp in range(2):
            for nh in range(2):
                psr = psvec.tile([4, 512], F32, tag="psv")
                nc.tensor.matmul(
                    psr[:],
                    lhsT=aab[:, c, hp, :],
                    rhs=tTb[:, hp, nh * 512:(nh + 1) * 512],
                    start=True,
                    stop=True,
                )
                nc.scalar.activation(
                    out=srcb[:, hp, nh * 512:(nh + 1) * 512], in_=psr[:],
                    func=AF.Copy,
                )
        nc.sync.dma_start(out=srd[c], in_=srcb[:])
        for hh in range(NH):
            make_sbc(c * NH + hh)

        # ---- A1: Hm with ones column -> H_aug[c] ----
        for nb in range(NB):
            ph = pshm.tile([128, 260], F32, tag="ph")
            nc.tensor.matmul(
                ph[:],
                lhsT=hTe[:, c, nb * 128:(nb + 1) * 128],
                rhs=we[:, c, :],
                start=True,
                stop=True,
            )
            nc.scalar.activation(
                out=H_aug[:, c, nb, :, :],
                in_=ph[:].rearrange("p (h o) -> p h o", h=NH),
                func=AF.Copy,
            )

        # ---- A5: Hbd[c] (b-side | d-side) ----
        for nb in range(NB):
            for h in range(NH):
                if (nb + h) % 2 == 0:
                    nc.gpsimd.tensor_tensor(
                        out=Hbd[:, c, nb, h, 0:65],
                        in0=H_aug[:, c, nb, h, :],
                        in1=bcol[:, h, nb:nb + 1].to_broadcast([128, 65]),
                        op=OP.mult,
                    )
                    nc.gpsimd.tensor_tensor(
                        out=Hbd[:, c, nb, h, 65:130],
                        in0=H_aug[:, c, nb, h, :],
                        in1=dcol[:, h, nb:nb + 1].to_broadcast([128, 65]),
                        op=OP.mult,
                    )
                else:
                    nc.scalar.activation(
                        out=Hbd[:, c, nb, h, 0:65], in_=H_aug[:, c, nb, h, :],
                        func=AF.Copy, scale=bcol[:, h, nb:nb + 1],
                    )
                    nc.scalar.activation(
                        out=Hbd[:, c, nb, h, 65:130], in_=H_aug[:, c, nb, h, :],
                        func=AF.Copy, scale=dcol[:, h, nb:nb + 1],
                    )

    # =================== stage B (8 global heads) ===================
    ov_l = {}


    def make_ov(c):
        # shared ov = V @ (d .* Haug); staged r-scaled per head as bf16
        rov = apool.tile([128, NB, NH, 65], BF16, tag=f"rov{c}")
        for ib in range(NB):
            pv = psov.tile([128, 260], F32, tag="pv")
            for jb in range(NB):
                nc.tensor.matmul(
                    pv[:],
                    lhsT=vT[:, jb, ib * 128:(ib + 1) * 128],
                    rhs=Hbd[:, c, jb, :, 65:130],
                    start=(jb == 0),
                    stop=(jb == NB - 1),
                )
            for hh in range(NH):
                nc.scalar.activation(
                    out=rov[:, ib, hh, :],
                    in_=pv[:, hh * 65:(hh + 1) * 65],
                    func=AF.Copy,
                    scale=rcol_sb[c][:, hh, ib:ib + 1],
                )
        ov_l[c] = rov

    vpt_l = {}

    def make_vpt(gh):
        c, h = gh // NH, gh % NH
        sbc = sbc_l.pop(gh)
        VpT = vppool.tile([128, NB, N], BF16, tag="vpt")
        for jb in range(NB):
            if MASK_MODE == "paged":
                nc.vector.tensor_paged_mask(
                    out=VpT[:, jb, :],
                    in_=vT[:, jb, :],
                    partition_indices=dstneg_l[gh][:, jb:jb + 1],
                    partition_step=0.0,
                    mask_offsets=sbc[:],
                )
            else:
                nc.vector.scalar_tensor_tensor(
                    out=VpT[:, jb, :],
                    in0=sbc[:],
                    scalar=dstneg_l[gh][:, jb:jb + 1],
                    in1=vT[:, jb, :],
                    op0=OP.is_ge,
                    op1=OP.mult,
                )
        vpt_l[gh] = VpT

    make_ov(0)
    make_vpt(0)

    for gh in range(C2 * NH):
        c, h = gh // NH, gh % NH
        if gh + 1 < 8:
            make_vpt(gh + 1)
        if gh == 3:
            make_ov(1)
        rov = ov_l[c]
        VpT = vpt_l.pop(gh)
        num = smallp.tile([128, NB, 65], F32, tag="num")
        for ib in range(NB):
            pp = pspp.tile([128, 130], F32, tag="pp")
            for jb in range(NB):
                nc.tensor.matmul(
                    pp[:],
                    lhsT=VpT[:, jb, ib * 128:(ib + 1) * 128],
                    rhs=Hbd[:, c, jb, h, :],
                    start=(jb == 0),
                    stop=(jb == NB - 1),
                )
            # inject r*ov into the b-side columns: pp[:, 0:65] += rov
            nc.tensor.matmul(
                pp[:, 0:65],
                lhsT=ident[:],
                rhs=rov[:, ib, h, :],
                start=False,
                stop=True,
                skip_group_check=True,
            )
            # nd = -r * ppd via ACT scaled copy (one PSUM read per engine op)
            nd = smallp.tile([128, 65], F32, tag="nd")
            nc.scalar.activation(
                out=nd[:], in_=pp[:, 65:130], func=AF.Copy,
                scale=negr_l[gh][:, ib:ib + 1],
            )
            # num = (ppb + r*ov) + nd
            nc.vector.tensor_tensor(
                out=num[:, ib, :], in0=nd[:], in1=pp[:, 0:65], op=OP.add,
            )
        stage = smallp.tile([128, NB, F], F32, tag="stage")
        if gh == C2 * NH - 1:
            # last head: fully incremental tail
            rec = vecp.tile([128, NB, 1], F32, tag="rec")
            for ib in range(NB):
                nc.vector.reciprocal(
                    out=rec[:, ib, :], in_=num[:, ib, 64:65]
                )
                nc.scalar.activation(
                    out=stage[:, ib, :],
                    in_=num[:, ib, 0:64],
                    func=AF.Copy,
                    scale=rec[:, ib, :],
                )
                nc.sync.dma_start(
                    out=out_ap[c, h][ib * 128:(ib + 1) * 128, :],
                    in_=stage[:, ib, :],
                )
        else:
            rec = vecp.tile([128, NB, 1], F32, tag="rec")
            nc.vector.reciprocal(out=rec[:], in_=num[:, :, 64:65])
            nc.gpsimd.tensor_tensor(
                out=stage[:],
                in0=num[:, :, 0:64],
                in1=rec[:].to_broadcast([128, NB, F]),
                op=OP.mult,
            )
            nc.sync.dma_start(
                out=out_ap[c, h].rearrange("(ib p) o -> p ib o", p=128),
                in_=stage[:],
            )


def _install_ntff_hook():
    """antenv.axon_hooks is missing in this image; inject an equivalent shim
    driving NTFF profiling via ctypes into libaxon_pjrt.so."""
    import types, ctypes, contextlib

    if "antenv.axon_hooks" in sys.modules:
        return
    so_path = "/opt/axon/libaxon_pjrt.so"
    try:
        lib = ctypes.CDLL(so_path)
        lib.axon_start_nrt_profile.argtypes = [
            ctypes.POINTER(ctypes.c_int64),
            ctypes.c_size_t,
        ]
        lib.axon_start_nrt_profile.restype = ctypes.c_int64
        lib.axon_stop_nrt_profile.argtypes = [ctypes.c_char_p]
        lib.axon_stop_nrt_profile.restype = ctypes.c_int64
    except (OSError, AttributeError):
        return

    @contextlib.contextmanager
    def _hook(output_dir, device_ids):
        import jax

        jax.devices()
        if device_ids:
            ids = (ctypes.c_int64 * len(device_ids))(*device_ids)
            rc = lib.axon_start_nrt_profile(ids, len(device_ids))
        else:
            rc = lib.axon_start_nrt_profile(None, 0)
        if rc != 0:
            raise RuntimeError(f"axon_start_nrt_profile rc={rc}")
        try:
            yield
        finally:
            n = lib.axon_stop_nrt_profile(str(output_dir).encode())
            print(f"profile: {n} file(s) written to {output_dir}", file=sys.stderr)

    mod = types.ModuleType("antenv.axon_hooks")
    mod.get_axon_ntff_profile_hook = lambda: _hook
    mod.set_axon_ntff_profile_hook = lambda h: None
    sys.modules["antenv.axon_hooks"] = mod

    import concourse.bass_utils as bu

    bu.upload_artifacts = lambda tmpdir: f"local:{tmpdir}"


_CACHED = {}


def _build_program():
    if "nc" in _CACHED:
        return _CACHED["nc"]
    nc = bacc.Bacc(
        "TRN2",
        target_bir_lowering=False,
        debug=False,
        enable_asserts=True,
        num_devices=8,
    )
    ins = {
        "vT": nc.dram_tensor("vT", [N, N], BF16, kind="ExternalInput").ap(),
        "hTe": nc.dram_tensor("hTe", [65, C2, N], BF16, kind="ExternalInput").ap(),
        "we": nc.dram_tensor(
            "we", [65, C2, NH * 65], BF16, kind="ExternalInput"
        ).ap(),
        "wb": nc.dram_tensor("wb", [64, C2, NH, F], BF16, kind="ExternalInput").ap(),
        "aab": nc.dram_tensor("aab", [128, C2, 2, 4], BF16, kind="ExternalInput").ap(),
    }
    out_ap = nc.dram_tensor(
        "out_loc", [C2, NH, N, F], F32, kind="ExternalOutput"
    ).ap()
    with tile.TileContext(nc) as tc:
        with ExitStack() as ctx:
            build_kernel(nc, tc, ctx, ins, out_ap)
    nc.compile()
    _CACHED["nc"] = nc
    return nc


def make_in_maps(h, adj, w, a_src, a_dst):
    bf = ml_dtypes.bfloat16
    eye = np.eye(N, dtype=np.float32)
    in_maps = []
    for core in range(8):
        b, cp = core // 2, core % 2
        cs = slice(2 * cp, 2 * cp + 2)
        # vT[j, i] = 1{adj[b][i, j] or i == j}
        vT = (((adj[b] + eye) > 0).astype(np.float32).T).astype(bf)
        # hTe: [65, 2, N]; rows 0:64 = h[b, c].T, row 64 = ones
        hTe = np.zeros((65, 2, N), np.float32)
        hTe[0:64] = h[b, cs].transpose(2, 0, 1)
        hTe[64] = 1.0
        # we: [65, 2, 4*65]: per head block 65 cols: w | e65
        we = np.zeros((65, 2, NH * 65), np.float32)
        for ci in range(2):
            for hh in range(NH):
                we[0:64, ci, hh * 65:hh * 65 + 64] = w[2 * cp + ci, hh]
                we[64, ci, hh * 65 + 64] = 1.0
        wv = np.ascontiguousarray(w[cs].transpose(2, 0, 1, 3))  # [64,2,4,64]
        # aab: [128, 2, 2, 4] block-diag (src_h0, -dst_h0, src_h1, -dst_h1)
        aab = np.zeros((128, 2, 2, 4), np.float32)
        for ci in range(2):
            for hp in range(2):
                aab[0:64, ci, hp, 0] = a_src[2 * cp + ci, 2 * hp, :, 0]
                aab[0:64, ci, hp, 1] = -a_dst[2 * cp + ci, 2 * hp, :, 0]
                aab[64:128, ci, hp, 2] = a_src[2 * cp + ci, 2 * hp + 1, :, 0]
                aab[64:128, ci, hp, 3] = -a_dst[2 * cp + ci, 2 * hp + 1, :, 0]
        in_maps.append(
            {
                "vT": np.ascontiguousarray(vT),
                "hTe": hTe.astype(bf),
                "we": we.astype(bf),
                "wb": wv.astype(bf),
                "aab": aab.astype(bf),
            }
        )
    return in_maps


def kernel(h, adj, w, a_src, a_dst, trace=False):
    h = np.asarray(h, np.float32)
    adj = np.asarray(adj, np.float32)
    w = np.asarray(w, np.float32)
    a_src = np.asarray(a_src, np.float32)
    a_dst = np.asarray(a_dst, np.float32)
    nc = _build_program()
    in_maps = make_in_maps(h, adj, w, a_src, a_dst)
    if trace:
        _install_ntff_hook()
    res = run_bass_kernel_spmd(nc, in_maps, list(range(8)), trace=trace)
    out = np.zeros((4, 4, 4, N, F), np.float32)
    for core in range(8):
        b, cp = core // 2, core % 2
        out[b, 2 * cp:2 * cp + 2] = res.results[core]["out_loc"]
    if trace:
        return out, res
    return out

